# revision 7
# baseline (speedup 1.0000x reference)
"""Trainium2 8-core kernel for biased-attention with sigmoid gating.

Reference computation (per batch b):
  q = heads(q_x @ Wq) * C**-0.5 ; k = heads(kv_x @ Wk) ; v = heads(kv_x @ Wv)
  a = softmax(q k^T + bias1 + bias2, axis=-1)
  o = (a @ v) gated by sigmoid(q_x @ Wg + bg), then @ Wo + bo

Shapes: B=2, Q=K=2048, CQ=CK=CV=256, H=8, C=32, CO=256.

Sharding: 8 cores = 2 batches x 4 query-quarters (512 rows each). Each core
computes all 8 heads for its rows; no cross-core communication is needed.

Design: the dominant HBM traffic is the two [B,H,Q,K] bias tensors; they are
pre-cast to bf16 on host (34 MB per core, ~95 us at the ~360 GB/s per-core
HBM ceiling) and laid out per-head as [128 k-part, 16 k-tile, 512 q] so each
2 MB DMA moves 16 KB contiguous runs per partition. The score plane is kept
entirely on the PE + Act engines so the slower DVE/GpSimd engines stay off
the critical path and the PE stays busy enough to hold its 2.4 GHz p-state:
  - per score tile the PE computes QK^T (start) and then accumulates both
    bias tiles into the same PSUM bank via identity matmuls (I @ B = B);
  - ScalarE applies exp straight out of PSUM (f32) into bf16;
  - the PE consumes exp(S^T) as the moving operand of the PV matmul;
  - V carries an extra all-ones column per head, so PV emits the softmax
    denominators for free; a tiny [33,128] PE back-transpose restores the
    natural orientation for the per-row normalization and gating.
Q/K are packed 4 heads per 128-partition tile (legal stationary bases
0/32/64/96 at contraction 32), halving K/Q-projection matmul rows.
"""

import numpy as np

B, Q, K, CQ, H, C, CO = 2, 2048, 2048, 256, 8, 32, 256
HC = H * C  # 256
QS = Q // 4  # 512 query rows per core
KT_N = K // 128  # 16 k-tiles
N_CORES = 8
SCALE = float(C) ** -0.5

_CACHED = {}


def _build():
    import concourse.bass as bass
    import concourse.mybir as mybir
    import concourse.tile as tile
    from concourse import bacc
    from concourse.masks import make_identity

    f32 = mybir.dt.float32
    bf16 = mybir.dt.bfloat16
    AF = mybir.ActivationFunctionType
    ALU = mybir.AluOpType

    nc = bacc.Bacc(None, target_bir_lowering=False)

    # activations arrive host-transposed and pre-cast to bf16: [C, rows]
    qxTd = nc.declare_dram_parameter("qxT", [CQ, QS], bf16, isOutput=False)
    kvxTd = nc.declare_dram_parameter("kvxT", [CQ, K], bf16, isOutput=False)
    # biases arrive host-transposed bf16: [H, 128 k-part, 16 k-tile, 512 q]
    b1 = nc.declare_dram_parameter("b1", [H, 128, KT_N, QS], bf16, isOutput=False)
    b2 = nc.declare_dram_parameter("b2", [H, 128, KT_N, QS], bf16, isOutput=False)
    # weights pre-cast to bf16 on host; Wq carries the C**-0.5 scale
    Wq = nc.declare_dram_parameter("Wq", [CQ, HC], bf16, isOutput=False)
    Wk = nc.declare_dram_parameter("Wk", [CQ, HC], bf16, isOutput=False)
    Wv = nc.declare_dram_parameter("Wv", [CQ, HC], bf16, isOutput=False)
    Wg = nc.declare_dram_parameter("Wg", [CQ, HC], bf16, isOutput=False)
    bg = nc.declare_dram_parameter("bg", [HC], f32, isOutput=False)
    Wo = nc.declare_dram_parameter("Wo", [HC, CO], bf16, isOutput=False)
    bo = nc.declare_dram_parameter("bo", [CO], f32, isOutput=False)
    out = nc.declare_dram_parameter("out", [QS, CO], f32, isOutput=True)

    with tile.TileContext(nc) as tc:
        with (
            tc.tile_pool(name="singles", bufs=1) as singles,
            tc.tile_pool(name="stage", bufs=3) as stage,
            tc.tile_pool(name="bias", bufs=1) as biasp,
            tc.tile_pool(name="work", bufs=3) as work,
            tc.tile_pool(name="ework", bufs=4) as ework,
            tc.tile_pool(name="ps", bufs=1, space="PSUM") as psp,
        ):
            ident = singles.tile([128, 128], bf16)
            make_identity(nc, ident)
            identf = singles.tile([128, 128], f32, tag="identf")
            make_identity(nc, identf)

            # ---- setup loads on the vector ring; projection critical path
            # (kvxT, Wk, Wq, qxT) first ----
            kvxT = singles.tile([128, 2, K], bf16, tag="kvxT")
            nc.scalar.dma_start(
                out=kvxT, in_=kvxTd[:, :].rearrange("(a p) k -> p a k", p=128)
            )
            wbf = {}
            for name, w in (("Wk", Wk), ("Wq", Wq)):
                wtile = singles.tile([128, 2, 256], bf16, tag=f"w_{name}")
                nc.scalar.dma_start(
                    out=wtile, in_=w[:, :].rearrange("(a p) c -> p a c", p=128)
                )
                wbf[name] = wtile
            qxT = singles.tile([128, 2, QS], bf16, tag="qxT")
            nc.scalar.dma_start(
                out=qxT, in_=qxTd[:, :].rearrange("(a p) q -> p a q", p=128)
            )
            for name, w in (("Wv", Wv), ("Wg", Wg), ("Wo", Wo)):
                wtile = singles.tile([128, 2, 256], bf16, tag=f"w_{name}")
                nc.scalar.dma_start(
                    out=wtile, in_=w[:, :].rearrange("(a p) c -> p a c", p=128)
                )
                wbf[name] = wtile
            bg_bc = singles.tile([128, HC], f32, tag="bg")
            nc.scalar.dma_start(out=bg_bc, in_=bg[:].partition_broadcast(128))
            bo_bc = singles.tile([128, CO], f32, tag="bo")
            nc.scalar.dma_start(out=bo_bc, in_=bo[:].partition_broadcast(128))

            # bias streams: one 2 MB DMA per (bias, head); b1 on the sync
            # ring, b2 on the gpsimd SWDGE ring. 3 heads buffered.
            BIAS_BUFS = 3
            b1_tiles = [None] * H
            b2_tiles = [None] * H

            def issue_bias(h):
                b1t = biasp.tile([128, KT_N, QS], bf16, tag="b1", bufs=BIAS_BUFS)
                nc.sync.dma_start(out=b1t, in_=b1[h])
                b1_tiles[h] = b1t
                b2t = biasp.tile([128, KT_N, QS], bf16, tag="b2", bufs=BIAS_BUFS)
                nc.gpsimd.dma_start(out=b2t, in_=b2[h])
                b2_tiles[h] = b2t

            for h in range(BIAS_BUFS):
                issue_bias(h)

            # Heads packed two per 128-partition tile at bases 0 and 32
            # (legal stationary bases); head h lives at partitions (h%2)*32
            # of pair slot h//2. Projections compute a pair per matmul (M=64).
            QT = singles.tile([128, H // 2, QS], bf16, tag="QT")
            KT = singles.tile([128, H // 2, K], bf16, tag="KT")

            for j in range(H // 2):
                cols = slice(j * 64, (j + 1) * 64)
                for kc in range(4):
                    ps = psp.tile([128, 512, 1], f32, tag="scores", bufs=4)
                    for ck in range(2):
                        nc.tensor.matmul(
                            ps[:64, :, 0],
                            wbf["Wk"][:, ck, cols],
                            kvxT[:, ck, kc * 512:(kc + 1) * 512],
                            start=(ck == 0),
                            stop=(ck == 1),
                        )
                    nc.vector.tensor_copy(
                        KT[:64, j, kc * 512:(kc + 1) * 512], ps[:64, :, 0]
                    )
                ps = psp.tile([128, QS, 1], f32, tag="scores", bufs=4)
                for ck in range(2):
                    nc.tensor.matmul(
                        ps[:64, :, 0],
                        wbf["Wq"][:, ck, cols],
                        qxT[:, ck, :],
                        start=(ck == 0),
                        stop=(ck == 1),
                    )
                nc.vector.tensor_copy(QT[:64, j, :], ps[:64, :, 0])

            # G natural [128q, 4qt, 256hc] f32 = sigmoid(qx @ Wg + bg),
            # computed before any exp so the act table loads only twice.
            Gn = singles.tile([128, 4, HC], f32, tag="Gn")
            for qt in range(4):
                ps = psp.tile([128, HC, 1], f32, tag="scores", bufs=4)
                for ck in range(2):
                    nc.tensor.matmul(
                        ps[:, :, 0],
                        qxT[:, ck, qt * 128:(qt + 1) * 128],
                        wbf["Wg"][:, ck, :],
                        start=(ck == 0),
                        stop=(ck == 1),
                    )
                gt = stage.tile([128, HC], f32, tag="gtmp")
                nc.vector.tensor_add(gt, ps[:, :, 0], bg_bc)
                nc.scalar.activation(Gn[:, qt, :], gt, AF.Sigmoid)

            # V natural [128kr, 16kt, 8h*33] bf16; per head 32 V columns plus
            # an all-ones column so the PV matmul emits softmax denominators
            # for free in output column 32. Drains on DVE (gpsimd can't read PSUM).
            Vn = singles.tile([128, KT_N, H * 33], bf16, tag="Vn")
            nc.gpsimd.memset(Vn, 1.0)
            for kt in range(KT_N):
                ps = psp.tile([128, HC, 1], f32, tag="scores", bufs=4)
                for ck in range(2):
                    nc.tensor.matmul(
                        ps[:, :, 0],
                        kvxT[:, ck, kt * 128:(kt + 1) * 128],
                        wbf["Wv"][:, ck, :],
                        start=(ck == 0),
                        stop=(ck == 1),
                    )
                for h in range(H):
                    nc.vector.tensor_copy(
                        Vn[:, kt, h * 33:h * 33 + 32], ps[:, h * 32:(h + 1) * 32, 0]
                    )

            # ---- main attention loops (transposed orientation) ----
            # Per score tile: PE computes QK^T (start), accumulates both bias
            # tiles via identity matmuls (stop), ScalarE applies exp straight
            # out of PSUM, and the PE consumes exp(S^T) as the moving operand
            # of the PV matmul (one-stage software pipeline on the PE ring).
            O_all = singles.tile([128, 4, HC], f32, tag="O_all")
            for h in range(H):
                hcol = h * 32
                base = (h % 2) * 32
                hsl = slice(base, base + 32)
                g = h // 2
                if h + BIAS_BUFS < H:
                    issue_bias(h + BIAS_BUFS)
                B1h = b1_tiles[h]
                B2h = b2_tiles[h]
                o_ps = psp.tile([33, QS, 1], f32, tag="o_acc", bufs=1)
                pv_pend = []
                for kt in range(KT_N):
                    s_ps = psp.tile([128, QS, 1], f32, tag="scores", bufs=4)
                    nc.tensor.matmul(
                        s_ps[:, :, 0],
                        KT[hsl, g, kt * 128:(kt + 1) * 128],
                        QT[hsl, g, :],
                        start=True,
                        stop=False,
                    )
                    nc.tensor.matmul(
                        s_ps[:, :, 0], ident, B1h[:, kt, :],
                        start=False, stop=False,
                    )
                    nc.tensor.matmul(
                        s_ps[:, :, 0], ident, B2h[:, kt, :],
                        start=False, stop=True,
                    )
                    et_sb = ework.tile([128, QS], bf16, tag="et")
                    nc.scalar.activation(et_sb, s_ps[:, :, 0], AF.Exp)
                    pv_pend.append((kt, et_sb))
                    if kt > 0:
                        pkt, pet = pv_pend.pop(0)
                        nc.tensor.matmul(
                            o_ps[:, :, 0],
                            Vn[:, pkt, hcol + h:hcol + h + 33],
                            pet,
                            start=(pkt == 0),
                            stop=False,
                        )
                pkt, pet = pv_pend.pop(0)
                nc.tensor.matmul(
                    o_ps[:, :, 0],
                    Vn[:, pkt, hcol + h:hcol + h + 33],
                    pet,
                    start=False,
                    stop=True,
                )
                oT_sb = work.tile([33, QS], f32, tag="oT")
                nc.vector.tensor_copy(oT_sb, o_ps[:, :, 0])
                for qt in range(4):
                    on_ps = psp.tile([128, C + 1, 1], f32, tag="onat", bufs=1)
                    nc.tensor.transpose(
                        on_ps[:, :, 0],
                        oT_sb[:, qt * 128:(qt + 1) * 128],
                        identf[:33, :33],
                    )
                    rinv = work.tile([128, 1], f32, tag="rinv")
                    nc.vector.reciprocal(rinv, on_ps[:, C:C + 1, 0])
                    nc.vector.tensor_scalar_mul(
                        O_all[:, qt, hcol:hcol + 32], on_ps[:, :C, 0], rinv
                    )

            # ---- gating + output projection ----
            for qt in range(4):
                og = stage.tile([128, HC], bf16, tag="og")
                nc.vector.tensor_mul(og, O_all[:, qt, :], Gn[:, qt, :])
                ogt_ps = psp.tile([128, 2, 128], bf16, tag="et_ps", bufs=2)
                for hcc in range(2):
                    nc.tensor.transpose(
                        ogt_ps[:, hcc, :], og[:, hcc * 128:(hcc + 1) * 128], ident
                    )
                ogt = stage.tile([128, 2, 128], bf16, tag="ogt")
                nc.vector.tensor_copy(ogt, ogt_ps)
                f_ps = psp.tile([128, CO, 1], f32, tag="scores", bufs=4)
                for hcc in range(2):
                    nc.tensor.matmul(
                        f_ps[:, :, 0],
                        ogt[:, hcc, :],
                        wbf["Wo"][:, hcc, :],
                        start=(hcc == 0),
                        stop=(hcc == 1),
                    )
                o_sb = stage.tile([128, CO], f32, tag="o_out")
                nc.vector.tensor_add(o_sb, f_ps[:, :, 0], bo_bc)
                nc.sync.dma_start(out=out[qt * 128:(qt + 1) * 128, :], in_=o_sb)

    nc.compile()
    return nc


def _get_nc():
    if "nc" not in _CACHED:
        _CACHED["nc"] = _build()
    return _CACHED["nc"]


def kernel(**inputs):
    from concourse.bass_utils import run_bass_kernel_spmd

    import ml_dtypes

    bf = ml_dtypes.bfloat16
    nc = _get_nc()
    inp = {k: np.asarray(v, dtype=np.float32) for k, v in inputs.items()}
    wq_b = (inp["Wq"] * SCALE).astype(bf)
    wk_b = inp["Wk"].astype(bf)
    wv_b = inp["Wv"].astype(bf)
    wg_b = inp["Wg"].astype(bf)
    wo_b = inp["Wo"].astype(bf)

    def bias_layout(x):
        # [H, QS, K] -> [H, 128 k-part, 16 k-tile, QS] bf16
        x = x.reshape(H, QS, KT_N, 128).transpose(0, 3, 2, 1)
        return np.ascontiguousarray(x).astype(bf)

    in_maps = []
    for c in range(N_CORES):
        b, qi = c // 4, c % 4
        q0 = qi * QS
        in_maps.append({
            "qxT": np.ascontiguousarray(inp["q_x"][b, q0:q0 + QS, :].T).astype(bf),
            "kvxT": np.ascontiguousarray(inp["kv_x"][b].T).astype(bf),
            "b1": bias_layout(inp["bias1"][b, :, q0:q0 + QS, :]),
            "b2": bias_layout(inp["bias2"][b, :, q0:q0 + QS, :]),
            "Wq": wq_b, "Wk": wk_b, "Wv": wv_b, "Wg": wg_b,
            "bg": inp["bg"], "Wo": wo_b, "bo": inp["bo"],
        })
    res = run_bass_kernel_spmd(nc, in_maps, core_ids=list(range(N_CORES)))
    outa = np.empty((B, Q, CO), np.float32)
    for c in range(N_CORES):
        b, qi = c // 4, c % 4
        outa[b, qi * QS:(qi + 1) * QS, :] = res.results[c]["out"]
    return outa


# revision 10
# speedup vs baseline: 1.1644x; 1.1644x over previous
"""Trainium2 8-core kernel for biased-attention with sigmoid gating.

Reference computation (per batch b):
  q = heads(q_x @ Wq) * C**-0.5 ; k = heads(kv_x @ Wk) ; v = heads(kv_x @ Wv)
  a = softmax(q k^T + bias1 + bias2, axis=-1)
  o = (a @ v) gated by sigmoid(q_x @ Wg + bg), then @ Wo + bo

Shapes: B=2, Q=K=2048, CQ=CK=CV=256, H=8, C=32, CO=256.

Sharding: 8 cores = 2 batches x 4 query-quarters (512 rows each). Each core
computes all 8 heads for its rows; no cross-core communication is needed.

Design: the dominant HBM traffic is the two [B,H,Q,K] bias tensors; they are
pre-cast to bf16 on host (34 MB per core, ~95 us at the ~360 GB/s per-core
HBM ceiling) and laid out per-head as [128 k-part, 16 k-tile, 512 q] so each
2 MB DMA moves 16 KB contiguous runs per partition. The score plane is kept
entirely on the PE + Act engines so the slower DVE/GpSimd engines stay off
the critical path and the PE stays busy enough to hold its 2.4 GHz p-state:
  - per score tile the PE computes QK^T (start) and then accumulates both
    bias tiles into the same PSUM bank via identity matmuls (I @ B = B);
  - ScalarE applies exp straight out of PSUM (f32) into bf16;
  - the PE consumes exp(S^T) as the moving operand of the PV matmul;
  - V carries an extra all-ones column per head, so PV emits the softmax
    denominators for free; a tiny [33,128] PE back-transpose restores the
    natural orientation for the per-row normalization and gating.
Q/K are packed 4 heads per 128-partition tile (legal stationary bases
0/32/64/96 at contraction 32), halving K/Q-projection matmul rows.
"""

import numpy as np

B, Q, K, CQ, H, C, CO = 2, 2048, 2048, 256, 8, 32, 256
HC = H * C  # 256
QS = Q // 4  # 512 query rows per core
KT_N = K // 128  # 16 k-tiles
N_CORES = 8
SCALE = float(C) ** -0.5

_CACHED = {}


def _build():
    import concourse.bass as bass
    import concourse.mybir as mybir
    import concourse.tile as tile
    from concourse import bacc
    from concourse.masks import make_identity

    f32 = mybir.dt.float32
    bf16 = mybir.dt.bfloat16
    AF = mybir.ActivationFunctionType
    ALU = mybir.AluOpType

    nc = bacc.Bacc(None, target_bir_lowering=False)

    # activations arrive host-transposed and pre-cast to bf16: [C, rows]
    qxTd = nc.declare_dram_parameter("qxT", [CQ, QS], bf16, isOutput=False)
    kvxTd = nc.declare_dram_parameter("kvxT", [CQ, K], bf16, isOutput=False)
    # biases arrive host-transposed bf16: [H, 128 k-part, 16 k-tile, 512 q]
    b1 = nc.declare_dram_parameter("b1", [H, 128, KT_N, QS], bf16, isOutput=False)
    b2 = nc.declare_dram_parameter("b2", [H, 128, KT_N, QS], bf16, isOutput=False)
    # weights pre-cast to bf16 on host; Wq carries the C**-0.5 scale
    Wq = nc.declare_dram_parameter("Wq", [CQ, HC], bf16, isOutput=False)
    Wk = nc.declare_dram_parameter("Wk", [CQ, HC], bf16, isOutput=False)
    Wv = nc.declare_dram_parameter("Wv", [CQ, HC], bf16, isOutput=False)
    Wg = nc.declare_dram_parameter("Wg", [CQ, HC], bf16, isOutput=False)
    bg = nc.declare_dram_parameter("bg", [HC], f32, isOutput=False)
    Wo = nc.declare_dram_parameter("Wo", [HC, CO], bf16, isOutput=False)
    bo = nc.declare_dram_parameter("bo", [CO], f32, isOutput=False)
    out = nc.declare_dram_parameter("out", [QS, CO], f32, isOutput=True)

    with tile.TileContext(nc) as tc:
        with (
            tc.tile_pool(name="singles", bufs=1) as singles,
            tc.tile_pool(name="stage", bufs=3) as stage,
            tc.tile_pool(name="bias", bufs=1) as biasp,
            tc.tile_pool(name="work", bufs=3) as work,
            tc.tile_pool(name="ework", bufs=4) as ework,
            tc.tile_pool(name="ps", bufs=1, space="PSUM") as psp,
        ):
            ident = singles.tile([128, 128], bf16)
            make_identity(nc, ident)
            identf = singles.tile([128, 128], f32, tag="identf")
            make_identity(nc, identf)

            # ---- setup loads on the vector ring; projection critical path
            # (kvxT, Wk, Wq, qxT) first ----
            kvxT = singles.tile([128, 2, K], bf16, tag="kvxT")
            nc.scalar.dma_start(
                out=kvxT, in_=kvxTd[:, :].rearrange("(a p) k -> p a k", p=128)
            )
            wbf = {}
            for name, w in (("Wk", Wk), ("Wq", Wq)):
                wtile = singles.tile([128, 2, 256], bf16, tag=f"w_{name}")
                nc.scalar.dma_start(
                    out=wtile, in_=w[:, :].rearrange("(a p) c -> p a c", p=128)
                )
                wbf[name] = wtile
            qxT = singles.tile([128, 2, QS], bf16, tag="qxT")
            nc.scalar.dma_start(
                out=qxT, in_=qxTd[:, :].rearrange("(a p) q -> p a q", p=128)
            )
            for name, w in (("Wv", Wv), ("Wg", Wg), ("Wo", Wo)):
                wtile = singles.tile([128, 2, 256], bf16, tag=f"w_{name}")
                nc.scalar.dma_start(
                    out=wtile, in_=w[:, :].rearrange("(a p) c -> p a c", p=128)
                )
                wbf[name] = wtile
            bg_bc = singles.tile([128, HC], f32, tag="bg")
            nc.scalar.dma_start(out=bg_bc, in_=bg[:].partition_broadcast(128))
            bo_bc = singles.tile([128, CO], f32, tag="bo")
            nc.scalar.dma_start(out=bo_bc, in_=bo[:].partition_broadcast(128))

            # bias streams: one 2 MB DMA per (bias, head); b1 on the sync
            # ring, b2 on the gpsimd SWDGE ring. 3 heads buffered.
            BIAS_BUFS = 3
            b1_tiles = [None] * H
            b2_tiles = [None] * H

            def issue_bias(h):
                b1t = biasp.tile([128, KT_N, QS], bf16, tag="b1", bufs=BIAS_BUFS)
                nc.sync.dma_start(out=b1t, in_=b1[h])
                b1_tiles[h] = b1t
                b2t = biasp.tile([128, KT_N, QS], bf16, tag="b2", bufs=BIAS_BUFS)
                nc.gpsimd.dma_start(out=b2t, in_=b2[h])
                b2_tiles[h] = b2t

            for h in range(BIAS_BUFS):
                issue_bias(h)

            # Heads packed two per 128-partition tile at bases 0 and 32
            # (legal stationary bases); head h lives at partitions (h%2)*32
            # of pair slot h//2. Projections compute a pair per matmul (M=64).
            QT = singles.tile([128, H // 2, QS], bf16, tag="QT")
            KT = singles.tile([128, H // 2, K], bf16, tag="KT")

            for j in range(H // 2):
                cols = slice(j * 64, (j + 1) * 64)
                for kc in range(4):
                    ps = psp.tile([128, 512, 1], f32, tag="scores", bufs=4)
                    for ck in range(2):
                        nc.tensor.matmul(
                            ps[:64, :, 0],
                            wbf["Wk"][:, ck, cols],
                            kvxT[:, ck, kc * 512:(kc + 1) * 512],
                            start=(ck == 0),
                            stop=(ck == 1),
                        )
                    nc.vector.tensor_copy(
                        KT[:64, j, kc * 512:(kc + 1) * 512], ps[:64, :, 0]
                    )
                ps = psp.tile([128, QS, 1], f32, tag="scores", bufs=4)
                for ck in range(2):
                    nc.tensor.matmul(
                        ps[:64, :, 0],
                        wbf["Wq"][:, ck, cols],
                        qxT[:, ck, :],
                        start=(ck == 0),
                        stop=(ck == 1),
                    )
                nc.vector.tensor_copy(QT[:64, j, :], ps[:64, :, 0])

            # G natural [128q, 4qt, 256hc] f32 = sigmoid(qx @ Wg + bg),
            # computed before any exp so the act table loads only twice.
            Gn = singles.tile([128, 4, HC], f32, tag="Gn")
            for qt in range(4):
                ps = psp.tile([128, HC, 1], f32, tag="scores", bufs=4)
                for ck in range(2):
                    nc.tensor.matmul(
                        ps[:, :, 0],
                        qxT[:, ck, qt * 128:(qt + 1) * 128],
                        wbf["Wg"][:, ck, :],
                        start=(ck == 0),
                        stop=(ck == 1),
                    )
                gt = stage.tile([128, HC], f32, tag="gtmp")
                nc.vector.tensor_add(gt, ps[:, :, 0], bg_bc)
                nc.scalar.activation(Gn[:, qt, :], gt, AF.Sigmoid)

            # V natural [128kr, 16kt, 8h*33] bf16; per head 32 V columns plus
            # an all-ones column so the PV matmul emits softmax denominators
            # for free in output column 32. Drains on DVE (gpsimd can't read PSUM).
            Vn = singles.tile([128, KT_N, H * 33], bf16, tag="Vn")
            nc.gpsimd.memset(Vn, 1.0)
            for kt in range(KT_N):
                ps = psp.tile([128, HC, 1], f32, tag="scores", bufs=4)
                for ck in range(2):
                    nc.tensor.matmul(
                        ps[:, :, 0],
                        kvxT[:, ck, kt * 128:(kt + 1) * 128],
                        wbf["Wv"][:, ck, :],
                        start=(ck == 0),
                        stop=(ck == 1),
                    )
                for h in range(H):
                    nc.vector.tensor_copy(
                        Vn[:, kt, h * 33:h * 33 + 32], ps[:, h * 32:(h + 1) * 32, 0]
                    )

            # ---- main attention loops (transposed orientation) ----
            # Per score tile: PE computes QK^T (start), accumulates both bias
            # tiles via identity matmuls (stop), ScalarE applies exp straight
            # out of PSUM, and the PE consumes exp(S^T) as the moving operand
            # of the PV matmul (one-stage software pipeline on the PE ring).
            O_all = singles.tile([128, 4, HC], f32, tag="O_all")
            norm_pend = []

            def emit_normalize():
                ph, oT_sb = norm_pend.pop(0)
                phcol = ph * 32
                for qt in range(4):
                    on_ps = psp.tile([128, C + 1, 1], f32, tag="onat", bufs=1)
                    nc.tensor.transpose(
                        on_ps[:, :, 0],
                        oT_sb[:, qt * 128:(qt + 1) * 128],
                        identf[:33, :33],
                    )
                    rinv = work.tile([128, 1], f32, tag="rinv")
                    nc.vector.reciprocal(rinv, on_ps[:, C:C + 1, 0])
                    nc.vector.tensor_scalar_mul(
                        O_all[:, qt, phcol:phcol + 32], on_ps[:, :C, 0], rinv
                    )

            for h in range(H):
                hcol = h * 32
                base = (h % 2) * 32
                hsl = slice(base, base + 32)
                g = h // 2
                B1h = b1_tiles[h]
                B2h = b2_tiles[h]
                o_ps = psp.tile([33, QS, 1], f32, tag="o_acc", bufs=2)
                pv_pend = []

                def emit_pv(last=False):
                    pkt, pet = pv_pend.pop(0)
                    nc.tensor.matmul(
                        o_ps[:, :, 0],
                        Vn[:, pkt, hcol + h:hcol + h + 33],
                        pet,
                        start=(pkt == 0),
                        stop=last,
                    )

                for kt in range(KT_N):
                    s_ps = psp.tile([128, QS, 1], f32, tag="scores", bufs=4)
                    nc.tensor.matmul(
                        s_ps[:, :, 0],
                        KT[hsl, g, kt * 128:(kt + 1) * 128],
                        QT[hsl, g, :],
                        start=True,
                        stop=False,
                    )
                    if kt % 8 < 5:
                        # gpsimd pre-sums b1+b2; one PE accumulate
                        Bs = ework.tile([128, QS], bf16, tag="bsum", bufs=6)
                        nc.gpsimd.tensor_tensor(
                            Bs, B1h[:, kt, :], B2h[:, kt, :], ALU.add
                        )
                        nc.tensor.matmul(
                            s_ps[:, :, 0], ident, Bs, start=False, stop=True,
                        )
                    else:
                        nc.tensor.matmul(
                            s_ps[:, :, 0], ident, B1h[:, kt, :],
                            start=False, stop=False,
                        )
                        nc.tensor.matmul(
                            s_ps[:, :, 0], ident, B2h[:, kt, :],
                            start=False, stop=True,
                        )
                    et_sb = ework.tile([128, QS], bf16, tag="et")
                    nc.scalar.activation(et_sb, s_ps[:, :, 0], AF.Exp)
                    pv_pend.append((kt, et_sb))
                    if len(pv_pend) > 2:
                        emit_pv()
                    if kt == 2 and norm_pend:
                        emit_normalize()
                while len(pv_pend) > 1:
                    emit_pv()
                emit_pv(last=True)
                oT_sb = work.tile([33, QS], f32, tag="oT", bufs=3)
                nc.vector.tensor_copy(oT_sb, o_ps[:, :, 0])
                norm_pend.append((h, oT_sb))
                # prefetch issued after this head's presums to avoid a
                # same-queue wait cycle on the gpsimd ring
                if h + BIAS_BUFS < H:
                    issue_bias(h + BIAS_BUFS)
            emit_normalize()

            # ---- gating + output projection ----
            for qt in range(4):
                og = stage.tile([128, HC], bf16, tag="og")
                nc.vector.tensor_mul(og, O_all[:, qt, :], Gn[:, qt, :])
                ogt_ps = psp.tile([128, 2, 128], bf16, tag="et_ps", bufs=1)
                for hcc in range(2):
                    nc.tensor.transpose(
                        ogt_ps[:, hcc, :], og[:, hcc * 128:(hcc + 1) * 128], ident
                    )
                ogt = stage.tile([128, 2, 128], bf16, tag="ogt")
                nc.vector.tensor_copy(ogt, ogt_ps)
                f_ps = psp.tile([128, CO, 1], f32, tag="scores", bufs=4)
                for hcc in range(2):
                    nc.tensor.matmul(
                        f_ps[:, :, 0],
                        ogt[:, hcc, :],
                        wbf["Wo"][:, hcc, :],
                        start=(hcc == 0),
                        stop=(hcc == 1),
                    )
                o_sb = stage.tile([128, CO], f32, tag="o_out")
                nc.vector.tensor_add(o_sb, f_ps[:, :, 0], bo_bc)
                nc.sync.dma_start(out=out[qt * 128:(qt + 1) * 128, :], in_=o_sb)

    nc.compile()
    return nc


def _get_nc():
    if "nc" not in _CACHED:
        _CACHED["nc"] = _build()
    return _CACHED["nc"]


def kernel(**inputs):
    from concourse.bass_utils import run_bass_kernel_spmd

    import ml_dtypes

    bf = ml_dtypes.bfloat16
    nc = _get_nc()
    inp = {k: np.asarray(v, dtype=np.float32) for k, v in inputs.items()}
    wq_b = (inp["Wq"] * SCALE).astype(bf)
    wk_b = inp["Wk"].astype(bf)
    wv_b = inp["Wv"].astype(bf)
    wg_b = inp["Wg"].astype(bf)
    wo_b = inp["Wo"].astype(bf)

    def bias_layout(x):
        # [H, QS, K] -> [H, 128 k-part, 16 k-tile, QS] bf16
        x = x.reshape(H, QS, KT_N, 128).transpose(0, 3, 2, 1)
        return np.ascontiguousarray(x).astype(bf)

    in_maps = []
    for c in range(N_CORES):
        b, qi = c // 4, c % 4
        q0 = qi * QS
        in_maps.append({
            "qxT": np.ascontiguousarray(inp["q_x"][b, q0:q0 + QS, :].T).astype(bf),
            "kvxT": np.ascontiguousarray(inp["kv_x"][b].T).astype(bf),
            "b1": bias_layout(inp["bias1"][b, :, q0:q0 + QS, :]),
            "b2": bias_layout(inp["bias2"][b, :, q0:q0 + QS, :]),
            "Wq": wq_b, "Wk": wk_b, "Wv": wv_b, "Wg": wg_b,
            "bg": inp["bg"], "Wo": wo_b, "bo": inp["bo"],
        })
    res = run_bass_kernel_spmd(nc, in_maps, core_ids=list(range(N_CORES)))
    outa = np.empty((B, Q, CO), np.float32)
    for c in range(N_CORES):
        b, qi = c // 4, c % 4
        outa[b, qi * QS:(qi + 1) * QS, :] = res.results[c]["out"]
    return outa
